# revision 12
# baseline (speedup 1.0000x reference)
"""Causal depthwise conv (kernel_size=4) on 8 TRN2 NeuronCores.

Problem: x (4, 4096, 16, 128) f32, weight (4, 16, 128) f32,
out[b,t,h,d] = sum_k weight[k,h,d] * x[b,t-k,h,d]   (zero-pad t<0).

Sharding: tensor-parallel over heads — core c owns heads [2c, 2c+2).
Host transposes each core's slice to d-major layout (partition dim = d,
free dim = t) and casts x to fp16: the correctness gate is rel err < 2e-2
against max|out| and fp16 end-to-end lands at ~7e-4, while halving HBM
traffic to 8.4 MB in + 8.4 MB out per core (~47 us at 358 GB/s).

Engine assignment (why not VectorE: scalar_tensor_tensor has no fast DVE
perf modes — 3 taps at 1x cost ~102 us/core, the previous bottleneck):

  TensorE  psum[d, t] += w_k[d] * x[d, t-k]   as 4 accumulating diagonal
           matmuls (lhsT = diag(w_k) fp16, rhs = shifted x window).
           One matmul per 512-col PSUM bank; ~131 ns/MM pipelined
           -> 256 MM/core ~ 34 us.
  ScalarE  evacuates PSUM (f32) -> SBUF fp16 out tile, 2048 cols/op
           -> ~32 us/core, overlapped with PE via 2 psum tiles.
  DMA      streams x in and out per stream (1.05 MB transfers) -> ~47 us,
           the critical path.  (Routing out-DMAs to the qAct HWDGE ring
           measured no steady-state win and ~2 ms extra per-dispatch cost,
           so everything stays on the qSP ring.)

Each stream ships with 3 zero columns prepended (causal pad), so taps never
cross stream boundaries.
"""

import time

import numpy as np

import concourse.mybir as mybir
from concourse import bacc, tile
from concourse.bass_utils import run_bass_kernel_spmd

BATCH, SEQ, N_HEADS, D_HEAD = 4, 4096, 16, 128
KERNEL = 4
PAD = KERNEL - 1
N_CORES = 8
H_PER_CORE = N_HEADS // N_CORES          # 2
N_STREAMS = H_PER_CORE * BATCH           # 8 per core; stream j = hl*BATCH + b
STRM = SEQ + PAD                         # 4099 columns per stream
CHUNK = 1                                # streams per input DMA
BANK = 512                               # f32 elems per PSUM bank
HALF = SEQ // 2                          # 2048 cols = 4 PSUM banks per tile

F32 = mybir.dt.float32
F16 = mybir.dt.float16

PROFILE = False          # set by test.py; adds a profiled run
TRACE_KWARGS = {}
last_exec_time_ns = None
last_results = None


def _build_module(
    chain: bool = False,
    repeats: int = 1,
    chunk: int = CHUNK,
    xbufs: int = 4,
    obufs: int = 4,
    out_split: int = 1,
    out_ring: str = "sync",
):
    """repeats>1 runs the whole kernel body that many times inside one NEFF
    (timing only).  `chain` kept for interface compat; unused.
    out_split: 1 = one out-DMA per stream, 2 = per half-stream."""
    nc = bacc.Bacc(
        "TRN2",
        target_bir_lowering=False,
        debug=False,
        num_devices=N_CORES,
        enable_asserts=False,
    )
    x = nc.dram_tensor("x", [D_HEAD, N_STREAMS, STRM], F16, kind="ExternalInput").ap()
    wd = nc.dram_tensor(
        "wd", [D_HEAD, H_PER_CORE * KERNEL * D_HEAD], F16, kind="ExternalInput"
    ).ap()
    out = nc.dram_tensor("out", [D_HEAD, N_STREAMS, SEQ], F16, kind="ExternalOutput").ap()
    out_dma = nc.scalar if out_ring == "scalar" else nc.sync

    with tile.TileContext(nc) as tc:
        with (
            tc.tile_pool(name="wp", bufs=1) as wp,
            tc.tile_pool(name="xp", bufs=xbufs) as xp,
            tc.tile_pool(name="op", bufs=obufs) as op,
            tc.psum_pool(name="pp", bufs=2) as pp,
        ):
            WD = wp.tile([D_HEAD, H_PER_CORE * KERNEL * D_HEAD], F16)
            nc.sync.dma_start(out=WD, in_=wd)
            for _r in range(repeats):
                for j0 in range(0, N_STREAMS, chunk):
                    X = xp.tile([D_HEAD, chunk * STRM], F16, tag="x")
                    nc.sync.dma_start(out=X, in_=x[:, j0 : j0 + chunk, :])
                    for s in range(chunk):
                        j = j0 + s
                        hl = j // BATCH
                        base = s * STRM
                        O = op.tile([D_HEAD, SEQ], F16, tag="o")
                        for half in range(2):
                            c0 = half * HALF
                            P = pp.tile([D_HEAD, HALF], F32, tag="p")
                            for cb in range(HALF // BANK):
                                r0 = base + PAD + c0 + cb * BANK
                                for k in range(KERNEL):
                                    D = WD[
                                        :,
                                        (hl * KERNEL + k) * D_HEAD : (hl * KERNEL + k + 1)
                                        * D_HEAD,
                                    ]
                                    nc.tensor.matmul(
                                        P[:, cb * BANK : (cb + 1) * BANK],
                                        D,
                                        X[:, r0 - k : r0 - k + BANK],
                                        start=(k == 0),
                                        stop=(k == KERNEL - 1),
                                    )
                            nc.scalar.activation(
                                O[:, c0 : c0 + HALF], P,
                                mybir.ActivationFunctionType.Copy,
                            )
                            if out_split == 2:
                                out_dma.dma_start(
                                    out=out[:, j, c0 : c0 + HALF],
                                    in_=O[:, c0 : c0 + HALF],
                                )
                        if out_split == 1:
                            out_dma.dma_start(out=out[:, j, :], in_=O)
    nc.compile()
    return nc


_module = None


def _get_module():
    global _module
    if _module is None:
        _module = _build_module()
    return _module


def _shard_inputs(x: np.ndarray, weight: np.ndarray):
    x16 = np.float16(x)                                  # (B, T, H, D)
    in_maps = []
    for c in range(N_CORES):
        h0 = c * H_PER_CORE
        xs = x16[:, :, h0 : h0 + H_PER_CORE, :]          # (B, T, HL, D)
        xt = np.ascontiguousarray(xs.transpose(3, 2, 0, 1))  # (D, HL, B, T)
        xin = np.zeros((D_HEAD, N_STREAMS, STRM), dtype=np.float16)
        xin[:, :, PAD:] = xt.reshape(D_HEAD, N_STREAMS, SEQ)
        # Diagonal weight blocks: lhsT[c', p] = w[k, h, p] * delta(c', p)
        wdm = np.zeros((D_HEAD, H_PER_CORE * KERNEL * D_HEAD), dtype=np.float16)
        for hl in range(H_PER_CORE):
            for k in range(KERNEL):
                blk = (hl * KERNEL + k) * D_HEAD
                wdm[:, blk : blk + D_HEAD] = np.diag(
                    weight[k, h0 + hl, :].astype(np.float16)
                )
        in_maps.append({"x": xin, "wd": wdm})
    return in_maps


def _unshard(results) -> np.ndarray:
    out = np.empty((BATCH, SEQ, N_HEADS, D_HEAD), dtype=np.float32)
    for c in range(N_CORES):
        h0 = c * H_PER_CORE
        o = results[c]["out"].reshape(D_HEAD, H_PER_CORE, BATCH, SEQ)
        out[:, :, h0 : h0 + H_PER_CORE, :] = o.transpose(2, 3, 1, 0)
    return out


def kernel(x: np.ndarray, weight: np.ndarray) -> np.ndarray:
    global last_exec_time_ns, last_results
    x = np.asarray(x, dtype=np.float32)
    weight = np.asarray(weight, dtype=np.float32)
    nc = _get_module()
    in_maps = _shard_inputs(x, weight)
    # The shared terminal occasionally wedges (NRT_EXEC_UNIT_UNRECOVERABLE)
    # and recovers after a pause; retry rather than fail the whole call.
    last_err = None
    for attempt in range(3):
        try:
            res = run_bass_kernel_spmd(
                nc, in_maps, list(range(N_CORES)), trace=PROFILE, **TRACE_KWARGS
            )
            break
        except Exception as e:  # noqa: BLE001 - device-transient errors
            last_err = e
            time.sleep(25 * (attempt + 1))
    else:
        raise last_err
    last_exec_time_ns = res.exec_time_ns
    last_results = res
    return _unshard(res.results)


# revision 14
# speedup vs baseline: 1.1500x; 1.1500x over previous
"""Causal depthwise conv (kernel_size=4) on 8 TRN2 NeuronCores.

Problem: x (4, 4096, 16, 128) f32, weight (4, 16, 128) f32,
out[b,t,h,d] = sum_k weight[k,h,d] * x[b,t-k,h,d]   (zero-pad t<0).

Sharding: tensor-parallel over heads — core c owns heads [2c, 2c+2).
Host transposes each core's slice to d-major layout (partition dim = d,
free dim = t) and casts x to fp16: the correctness gate is rel err < 2e-2
against max|out| and fp16 end-to-end lands at ~7e-4, while halving HBM
traffic to 8.4 MB in + 8.4 MB out per core (~47 us at 358 GB/s).

Engine assignment (why not VectorE: scalar_tensor_tensor has no fast DVE
perf modes — 3 taps at 1x cost ~102 us/core, the previous bottleneck):

  TensorE  psum[d, t] += w_k[d] * x[d, t-k]   as 4 accumulating diagonal
           matmuls (lhsT = diag(w_k) fp16, rhs = shifted x window).
           One matmul per 512-col PSUM bank; ~131 ns/MM pipelined
           -> 256 MM/core ~ 34 us.
  ScalarE  evacuates PSUM (f32) -> SBUF fp16 out tile, 2048 cols/op
           -> ~32 us/core, overlapped with PE via 2 psum tiles.
  DMA      streams x in and out per stream (1.05 MB transfers) -> ~47 us,
           the critical path.  (Routing out-DMAs to the qAct HWDGE ring
           measured no steady-state win and ~2 ms extra per-dispatch cost,
           so everything stays on the qSP ring.)

Each stream ships with 3 zero columns prepended (causal pad), so taps never
cross stream boundaries.
"""

import time

import numpy as np

import concourse.mybir as mybir
from concourse import bacc, tile
from concourse.bass_utils import run_bass_kernel_spmd

BATCH, SEQ, N_HEADS, D_HEAD = 4, 4096, 16, 128
KERNEL = 4
PAD = KERNEL - 1
N_CORES = 8
H_PER_CORE = N_HEADS // N_CORES          # 2
N_STREAMS = H_PER_CORE * BATCH           # 8 per core; stream j = hl*BATCH + b
STRM = SEQ + PAD                         # 4099 columns per stream
CHUNK = 1                                # streams per input DMA
BANK = 512                               # f32 elems per PSUM bank
HALF = SEQ // 2                          # 2048 cols = 4 PSUM banks per tile

F32 = mybir.dt.float32
F16 = mybir.dt.float16

PROFILE = False          # set by test.py; adds a profiled run
TRACE_KWARGS = {}
last_exec_time_ns = None
last_results = None


def _build_module(
    chain: bool = False,
    repeats: int = 1,
    chunk: int = CHUNK,
    xbufs: int = 4,
    obufs: int = 4,
    out_split: int = 1,
    out_ring: str = "sync",
    out_batch: int = 1,
):
    """repeats>1 runs the whole kernel body that many times inside one NEFF
    (timing only).  `chain` kept for interface compat; unused.
    out_split: 1 = one out-DMA per stream, 2 = per half-stream."""
    nc = bacc.Bacc(
        "TRN2",
        target_bir_lowering=False,
        debug=False,
        num_devices=N_CORES,
        enable_asserts=False,
    )
    x = nc.dram_tensor("x", [D_HEAD, N_STREAMS, STRM], F16, kind="ExternalInput").ap()
    wd = nc.dram_tensor(
        "wd", [D_HEAD, H_PER_CORE * KERNEL * D_HEAD], F16, kind="ExternalInput"
    ).ap()
    out = nc.dram_tensor("out", [D_HEAD, N_STREAMS, SEQ], F16, kind="ExternalOutput").ap()
    out_dma = nc.scalar if out_ring == "scalar" else nc.sync

    with tile.TileContext(nc) as tc:
        with (
            tc.tile_pool(name="wp", bufs=1) as wp,
            tc.tile_pool(name="xp", bufs=xbufs) as xp,
            tc.tile_pool(name="op", bufs=obufs) as op,
            tc.psum_pool(name="pp", bufs=2) as pp,
        ):
            WD = wp.tile([D_HEAD, H_PER_CORE * KERNEL * D_HEAD], F16)
            nc.sync.dma_start(out=WD, in_=wd)
            assert chunk % out_batch == 0
            for _r in range(repeats):
                for j0 in range(0, N_STREAMS, chunk):
                    X = xp.tile([D_HEAD, chunk * STRM], F16, tag="x")
                    nc.sync.dma_start(out=X, in_=x[:, j0 : j0 + chunk, :])
                    for g0 in range(0, chunk, out_batch):
                        O = op.tile([D_HEAD, out_batch * SEQ], F16, tag="o")
                        for s in range(g0, g0 + out_batch):
                            j = j0 + s
                            hl = j // BATCH
                            base = s * STRM
                            o0 = (s - g0) * SEQ
                            for half in range(2):
                                c0 = half * HALF
                                P = pp.tile([D_HEAD, HALF], F32, tag="p")
                                for cb in range(HALF // BANK):
                                    r0 = base + PAD + c0 + cb * BANK
                                    for k in range(KERNEL):
                                        D = WD[
                                            :,
                                            (hl * KERNEL + k) * D_HEAD
                                            : (hl * KERNEL + k + 1) * D_HEAD,
                                        ]
                                        nc.tensor.matmul(
                                            P[:, cb * BANK : (cb + 1) * BANK],
                                            D,
                                            X[:, r0 - k : r0 - k + BANK],
                                            start=(k == 0),
                                            stop=(k == KERNEL - 1),
                                        )
                                nc.scalar.activation(
                                    O[:, o0 + c0 : o0 + c0 + HALF], P,
                                    mybir.ActivationFunctionType.Copy,
                                )
                                if out_split == 2:
                                    out_dma.dma_start(
                                        out=out[:, j, c0 : c0 + HALF],
                                        in_=O[:, o0 + c0 : o0 + c0 + HALF],
                                    )
                        if out_split == 1:
                            out_dma.dma_start(
                                out=out[:, j0 + g0 : j0 + g0 + out_batch, :], in_=O
                            )
    nc.compile()
    return nc


_module = None


def _get_module():
    global _module
    if _module is None:
        _module = _build_module()
    return _module


def _shard_inputs(x: np.ndarray, weight: np.ndarray):
    x16 = np.float16(x)                                  # (B, T, H, D)
    in_maps = []
    for c in range(N_CORES):
        h0 = c * H_PER_CORE
        xs = x16[:, :, h0 : h0 + H_PER_CORE, :]          # (B, T, HL, D)
        xt = np.ascontiguousarray(xs.transpose(3, 2, 0, 1))  # (D, HL, B, T)
        xin = np.zeros((D_HEAD, N_STREAMS, STRM), dtype=np.float16)
        xin[:, :, PAD:] = xt.reshape(D_HEAD, N_STREAMS, SEQ)
        # Diagonal weight blocks: lhsT[c', p] = w[k, h, p] * delta(c', p)
        wdm = np.zeros((D_HEAD, H_PER_CORE * KERNEL * D_HEAD), dtype=np.float16)
        for hl in range(H_PER_CORE):
            for k in range(KERNEL):
                blk = (hl * KERNEL + k) * D_HEAD
                wdm[:, blk : blk + D_HEAD] = np.diag(
                    weight[k, h0 + hl, :].astype(np.float16)
                )
        in_maps.append({"x": xin, "wd": wdm})
    return in_maps


def _unshard(results) -> np.ndarray:
    out = np.empty((BATCH, SEQ, N_HEADS, D_HEAD), dtype=np.float32)
    for c in range(N_CORES):
        h0 = c * H_PER_CORE
        o = results[c]["out"].reshape(D_HEAD, H_PER_CORE, BATCH, SEQ)
        out[:, :, h0 : h0 + H_PER_CORE, :] = o.transpose(2, 3, 1, 0)
    return out


def kernel(x: np.ndarray, weight: np.ndarray) -> np.ndarray:
    global last_exec_time_ns, last_results
    x = np.asarray(x, dtype=np.float32)
    weight = np.asarray(weight, dtype=np.float32)
    nc = _get_module()
    in_maps = _shard_inputs(x, weight)
    # The shared terminal occasionally wedges (NRT_EXEC_UNIT_UNRECOVERABLE)
    # and recovers after a pause; retry rather than fail the whole call.
    last_err = None
    for attempt in range(3):
        try:
            res = run_bass_kernel_spmd(
                nc, in_maps, list(range(N_CORES)), trace=PROFILE, **TRACE_KWARGS
            )
            break
        except Exception as e:  # noqa: BLE001 - device-transient errors
            last_err = e
            time.sleep(25 * (attempt + 1))
    else:
        raise last_err
    last_exec_time_ns = res.exec_time_ns
    last_results = res
    return _unshard(res.results)
